# revision 15
# baseline (speedup 1.0000x reference)
"""CBOW negative-sampling loss kernel for Trainium2 (8 NeuronCores, SPMD).

Computes, for full inputs:
    pos_u_emb = sum_c u_weight[pos_u[:, c]]          # [B, E]
    neg_u_emb = sum_c u_weight[neg_u[:, c]]          # [B, E]
    p = rowdot(pos_u_emb, w_weight[pos_w])           # [B]
    n = rowdot(neg_u_emb, w_weight[neg_w])           # [B]
    loss = -(sum(log_sigmoid(p)) + sum(log_sigmoid(-n)))

Strategy: data-parallel over the batch (2048 rows per core); both embedding
tables are replicated per core as one concatenated [2*TABLE, E] fp8e4 tensor,
pre-scaled by 2^10 so the values sit in fp8's normal range (the loss is
dominated by 2*B*ln2; score noise from fp8 is ~1e-6 relative). fp8 halves
the gather bytes vs bf16 (256 B/row). Host packs, per 128-row tile, the 10
context-row indices plus the (offset) target-row index as 11 int32s per
batch row; one indirect (gather) DMA per 4 tiles fetches 5632 rows.

The 10-row context sum runs on the otherwise-idle PE as 10 identity-matmuls
accumulating into PSUM (PE reads fp8 natively; DVE would be 1x on fp8).
A single DVE tensor_tensor_reduce then multiplies the PSUM sum by the
target row and row-reduces into per-tile scores; ACT computes softplus via
Exp/Ln with the 2^-20 descale riding on Exp's scale input. Each core emits
128 per-partition partials; the host sums 8*128 floats.
"""

import sys

sys.path.insert(0, "/opt/trn_rl_repo")

import numpy as np

import concourse.bacc as bacc
import concourse.bass as bass
import concourse.mybir as mybir
import concourse.tile as tile

P = 128
EMB = 256
TABLE = 199999
CTX = 10
K = CTX + 1
B = 16384
N_CORES = 8
B_CORE = B // N_CORES  # 2048
N_TILES = B_CORE // P  # 16

TABLE_SCALE = 1024.0  # 2^10: lifts fp8e4 values into normal range
SCORE_DESCALE = 1.0 / (TABLE_SCALE * TABLE_SCALE)  # 2^-20

_NC_CACHE = {}
LAST_RESULT = None  # BassKernelResults of the most recent kernel() call


def build_nc(
    table_rows=2 * TABLE,
    emb=EMB,
    k=K,
    n_tiles=N_TILES,
    g_bufs=3,
    finalize=True,
    reps=1,
    table_dt=mybir.dt.float8e4,
    mode="full",  # "full" | "dma_only" | "compute_only" (timing probes)
    tiles_per_gather=4,
    use_ttr=False,  # InstTensorTensorReduce faults on this HW path
    use_pe=True,
    double_row=True,  # PE DoubleRow fp8: one matmul sums 2 ctx rows
):
    """Build the per-core Bass module (same program for every core)."""
    nc = bacc.Bacc(
        "TRN2",
        target_bir_lowering=False,
        debug=False,
        num_devices=N_CORES,
    )
    table = nc.declare_dram_parameter(
        "table", [table_rows, emb], table_dt, isOutput=False
    )
    pos_idx = nc.declare_dram_parameter(
        "pos_idx", [P, n_tiles * k], mybir.dt.int32, isOutput=False
    )
    neg_idx = nc.declare_dram_parameter(
        "neg_idx", [P, n_tiles * k], mybir.dt.int32, isOutput=False
    )
    ident = nc.declare_dram_parameter("ident", [P, 2 * P], table_dt, isOutput=False)
    out = nc.declare_dram_parameter("out", [P], mybir.dt.float32, isOutput=True)

    ctx = CTX if k == CTX + 1 else k - 1

    with tile.TileContext(nc) as tc:
        with (
            tc.tile_pool(name="idx", bufs=1) as idxp,
            tc.tile_pool(name="g", bufs=g_bufs) as gp,
            tc.psum_pool(name="ps", bufs=4) as psp,
            tc.tile_pool(name="scr", bufs=2) as scrp,
            tc.tile_pool(name="s", bufs=1) as sp,
        ):
            IP = idxp.tile([P, n_tiles * k], mybir.dt.int32, tag="ip")
            IN = idxp.tile([P, n_tiles * k], mybir.dt.int32, tag="in")
            IDT = idxp.tile([P, 2 * P], table_dt, tag="ident")
            ID = IDT[:, 0:P]
            ID2 = IDT[:, :].rearrange("p (two m) -> p two m", two=2)
            nc.sync.dma_start(out=IP[:], in_=pos_idx[:])
            nc.sync.dma_start(out=IN[:], in_=neg_idx[:])
            nc.sync.dma_start(out=IDT[:], in_=ident[:])

            S_pos = sp.tile([P, n_tiles], mybir.dt.float32, tag="spos")
            S_neg = sp.tile([P, n_tiles], mybir.dt.float32, tag="sneg")
            if mode == "dma_only":
                nc.vector.memset(S_pos[:], 0.0)
                nc.vector.memset(S_neg[:], 0.0)

            tpg = tiles_per_gather
            assert n_tiles % tpg == 0
            for _rep in range(reps):
              for t in range(n_tiles // tpg):
                for S, IDX in ((S_pos, IP), (S_neg, IN)):
                    if mode == "compute_only":
                        if "G_static" not in locals():
                            G_static = gp.tile([P, tpg * k * emb], table_dt, tag="g")
                            nc.gpsimd.indirect_dma_start(
                                out=G_static[:],
                                out_offset=None,
                                in_=table[:],
                                in_offset=bass.IndirectOffsetOnAxis(
                                    ap=IDX[:, 0 : tpg * k], axis=0
                                ),
                            )
                        G = G_static
                    else:
                        G = gp.tile([P, tpg * k * emb], table_dt, tag="g")
                        nc.gpsimd.indirect_dma_start(
                            out=G[:],
                            out_offset=None,
                            in_=table[:],
                            in_offset=bass.IndirectOffsetOnAxis(
                                ap=IDX[:, t * tpg * k : (t + 1) * tpg * k], axis=0
                            ),
                        )
                    if mode == "dma_only":
                        continue
                    for j in range(tpg):
                        gt = t * tpg + j  # global tile index
                        base = j * k * emb
                        if use_pe:
                            # context sum on PE: identity-matmuls -> PSUM.
                            # DoubleRow fp8 sums 2 consecutive ctx rows per
                            # matmul: out = ID.T@rhs[:, :E] + ID.T@rhs[:, E:].
                            PU = psp.tile(
                                [P, emb], mybir.dt.float32, tag="pu", space="PSUM"
                            )
                            if double_row:
                                assert ctx % 2 == 0
                                for h in range(ctx // 2):
                                    rhs = G[
                                        :, base + 2 * h * emb : base + (2 * h + 2) * emb
                                    ].rearrange("p (two e) -> p two e", two=2)
                                    nc.tensor.matmul(
                                        PU[:],
                                        ID2,
                                        rhs,
                                        start=(h == 0),
                                        stop=(h == ctx // 2 - 1),
                                        perf_mode=mybir.MatmulPerfMode.DoubleRow,
                                    )
                            else:
                                for c in range(ctx):
                                    nc.tensor.matmul(
                                        PU[:],
                                        ID,
                                        G[:, base + c * emb : base + (c + 1) * emb],
                                        start=(c == 0),
                                        stop=(c == ctx - 1),
                                    )
                        else:
                            # DVE pairwise add tree (bisect fallback; slow on fp8)
                            A = scrp.tile([P, 5 * emb], table_dt, tag="a")
                            PU = scrp.tile([P, emb], table_dt, tag="puv")
                            add = mybir.AluOpType.add
                            nc.vector.tensor_tensor(
                                out=A[:],
                                in0=G[:, base : base + 5 * emb],
                                in1=G[:, base + 5 * emb : base + 10 * emb],
                                op=add,
                            )
                            nc.vector.tensor_tensor(
                                out=A[:, 0 : 2 * emb],
                                in0=A[:, 0 : 2 * emb],
                                in1=A[:, 2 * emb : 4 * emb],
                                op=add,
                            )
                            nc.vector.tensor_tensor(
                                out=PU[:], in0=A[:, 0:emb], in1=A[:, emb : 2 * emb], op=add
                            )
                            nc.vector.tensor_tensor(
                                out=PU[:], in0=PU[:], in1=A[:, 4 * emb : 5 * emb], op=add
                            )
                        if use_ttr:
                            # fused (PU * w_row) + row-reduce on DVE
                            DOT = scrp.tile([P, emb], mybir.dt.bfloat16, tag="dot")
                            nc.vector.tensor_tensor_reduce(
                                out=DOT[:],
                                in0=PU[:],
                                in1=G[:, base + ctx * emb : base + k * emb],
                                scale=1.0,
                                scalar=0.0,
                                op0=mybir.AluOpType.mult,
                                op1=mybir.AluOpType.add,
                                accum_out=S[:, gt : gt + 1],
                            )
                        else:
                            DOT = scrp.tile([P, emb], mybir.dt.bfloat16, tag="dot")
                            nc.vector.tensor_tensor(
                                out=DOT[:],
                                in0=PU[:],
                                in1=G[:, base + ctx * emb : base + k * emb],
                                op=mybir.AluOpType.mult,
                            )
                            JD = scrp.tile([P, emb], mybir.dt.float32, tag="jd")
                            nc.scalar.activation(
                                out=JD[:],
                                in_=DOT[:],
                                func=mybir.ActivationFunctionType.Copy,
                                accum_out=S[:, gt : gt + 1],
                            )

            # -log_sigmoid(p) = softplus(-p) = ln(1 + exp(-p));
            # -log_sigmoid(-n) = softplus(n) = ln(1 + exp(n)).
            # Scores carry the 2^20 table scale; descale on Exp's scale input.
            EP = sp.tile([P, n_tiles], mybir.dt.float32, tag="ep")
            EN = sp.tile([P, n_tiles], mybir.dt.float32, tag="en")
            JUNK = sp.tile([P, n_tiles], mybir.dt.float32, tag="junk")
            ACC = sp.tile([P, 2], mybir.dt.float32, tag="acc")
            nc.scalar.activation(
                out=EP[:],
                in_=S_pos[:],
                func=mybir.ActivationFunctionType.Exp,
                scale=-SCORE_DESCALE,
            )
            nc.scalar.activation(
                out=JUNK[:],
                in_=EP[:],
                func=mybir.ActivationFunctionType.Ln,
                bias=1.0,
                accum_out=ACC[:, 0:1],
            )
            nc.scalar.activation(
                out=EN[:],
                in_=S_neg[:],
                func=mybir.ActivationFunctionType.Exp,
                scale=SCORE_DESCALE,
            )
            nc.scalar.activation(
                out=JUNK[:],
                in_=EN[:],
                func=mybir.ActivationFunctionType.Ln,
                bias=1.0,
                accum_out=ACC[:, 1:2],
            )
            V = sp.tile([P, 1], mybir.dt.float32, tag="v")
            nc.vector.tensor_tensor(
                out=V[:],
                in0=ACC[:, 0:1],
                in1=ACC[:, 1:2],
                op=mybir.AluOpType.add,
            )
            nc.sync.dma_start(out=out[:], in_=V[:, 0])
    if finalize:
        nc.finalize()
    return nc


def _prep_idx(u_idx, w_idx, n_tiles):
    """[Bc, CTX] + [Bc] -> [P, n_tiles*K] int32 in the tile-major layout the
    kernel expects: partition p, cols t*K:(t+1)*K hold row t*P+p's indices,
    with the w index (offset into the second table half) last."""
    k = u_idx.shape[1] + 1
    a = np.concatenate(
        [u_idx.astype(np.int64), (w_idx.astype(np.int64) + TABLE)[:, None]], axis=1
    ).astype(np.int32)
    return np.ascontiguousarray(
        a.reshape(n_tiles, P, k).transpose(1, 0, 2).reshape(P, n_tiles * k)
    )


def _prep_table(u_weight, w_weight):
    table_np_dt = mybir.dt.np(mybir.dt.float8e4)
    return np.ascontiguousarray(
        (
            np.concatenate(
                [
                    np.asarray(u_weight, np.float32),
                    np.asarray(w_weight, np.float32),
                ],
                axis=0,
            )
            * TABLE_SCALE
        ).astype(table_np_dt)
    )


def _prep_ident():
    eye = np.eye(P, dtype=mybir.dt.np(mybir.dt.float8e4))
    return np.ascontiguousarray(np.concatenate([eye, eye], axis=1))


def make_in_maps(pos_u, pos_w, neg_u, neg_w, u_weight, w_weight):
    pos_u = np.asarray(pos_u)
    pos_w = np.asarray(pos_w)
    neg_u = np.asarray(neg_u)
    neg_w = np.asarray(neg_w)

    table = _prep_table(u_weight, w_weight)
    ident = _prep_ident()

    in_maps = []
    for c in range(N_CORES):
        sl = slice(c * B_CORE, (c + 1) * B_CORE)
        in_maps.append(
            {
                "table": table,
                "ident": ident,
                "pos_idx": _prep_idx(pos_u[sl], pos_w[sl], N_TILES),
                "neg_idx": _prep_idx(neg_u[sl], neg_w[sl], N_TILES),
            }
        )
    return in_maps


def kernel(pos_u, pos_w, neg_u, neg_w, u_weight, w_weight):
    from concourse.bass_utils import run_bass_kernel_spmd

    if "nc" not in _NC_CACHE:
        _NC_CACHE["nc"] = build_nc()
    nc = _NC_CACHE["nc"]

    in_maps = make_in_maps(pos_u, pos_w, neg_u, neg_w, u_weight, w_weight)

    global LAST_RESULT
    res = run_bass_kernel_spmd(nc, in_maps, list(range(N_CORES)))
    LAST_RESULT = res
    total = 0.0
    for r in res.results:
        total += float(np.sum(r["out"].astype(np.float64)))
    return np.float32(total)


# revision 24
# speedup vs baseline: 1.5128x; 1.5128x over previous
"""CBOW negative-sampling loss kernel for Trainium2 (8 NeuronCores, SPMD).

Computes, for full inputs:
    pos_u_emb = sum_c u_weight[pos_u[:, c]]          # [B, E]
    neg_u_emb = sum_c u_weight[neg_u[:, c]]          # [B, E]
    p = rowdot(pos_u_emb, w_weight[pos_w])           # [B]
    n = rowdot(neg_u_emb, w_weight[neg_w])           # [B]
    loss = -(sum(log_sigmoid(p)) + sum(log_sigmoid(-n)))

Strategy: data-parallel over the batch (2048 rows per core); both embedding
tables are replicated per core as one concatenated [2*TABLE, E] fp8e4 tensor,
pre-scaled by 2^10 so the values sit in fp8's normal range (the loss is
dominated by 2*B*ln2; fp8 score noise is ~1e-6 relative). fp8 halves the
gather bytes vs bf16 (256 B/row).

Per 128-row tile the host packs 22 int32 indices per batch row:
[10 pos ctx, pos w+TABLE, 10 neg ctx, neg w+TABLE]; one indirect gather per
2 tiles fetches 5632 rows. The 10-row context sums run on the otherwise-idle
PE as identity-matmuls accumulating into PSUM (PE reads fp8 natively; DVE
would be 1x on fp8): 4 groups (2 tiles x pos/neg) share one [128,1024] PSUM
tile. One DVE tensor_tensor mult (1x: PSUM fp32 operand) then forms all 4
DOT chunks against the strided target-row view of G; ACT row-reduces each
chunk into per-tile scores (Copy+accum) and computes softplus via Exp/Ln
with the 2^-20 descale riding on Exp's scale input. Each core emits 128
per-partition partials; the host sums 8*128 floats.
"""

import sys

sys.path.insert(0, "/opt/trn_rl_repo")

import numpy as np

import concourse.bacc as bacc
import concourse.bass as bass
import concourse.mybir as mybir
import concourse.tile as tile

P = 128
EMB = 256
TABLE = 199999
CTX = 10
K = CTX + 1
K2 = 2 * K  # pos+neg packed per tile
B = 16384
N_CORES = 8
B_CORE = B // N_CORES  # 2048
N_TILES = B_CORE // P  # 16

TABLE_SCALE = 1024.0  # 2^10: lifts fp8e4 values into normal range
SCORE_DESCALE = 1.0 / (TABLE_SCALE * TABLE_SCALE)  # 2^-20

_NC_CACHE = {}
LAST_RESULT = None  # BassKernelResults of the most recent kernel() call


def build_nc(
    table_rows=2 * TABLE,
    emb=EMB,
    n_tiles=N_TILES,
    g_bufs=3,
    finalize=True,
    reps=1,
    table_dt=mybir.dt.float8e4,
    mode="full",  # "full" | "dma_only" | "compute_only" (timing probes)
    tiles_per_gather=2,
    dve_reduce=0,  # probe: this many of the 4 row-reduces per group on DVE
    pe_ctx=None,  # probe: only this many ctx matmuls (wrong result)
    skip_mult=False,  # probe: skip the DVE mult, ACT reduces PU (wrong result)
):
    """Build the per-core Bass module (same program for every core)."""
    nc = bacc.Bacc(
        "TRN2",
        target_bir_lowering=False,
        debug=False,
        num_devices=N_CORES,
    )
    table = nc.declare_dram_parameter(
        "table", [table_rows, emb], table_dt, isOutput=False
    )
    idx_in = nc.declare_dram_parameter(
        "idx", [P, n_tiles * K2], mybir.dt.int32, isOutput=False
    )
    ident = nc.declare_dram_parameter("ident", [P, P], table_dt, isOutput=False)
    out = nc.declare_dram_parameter("out", [P], mybir.dt.float32, isOutput=True)

    ctx = CTX
    tpg = tiles_per_gather
    assert n_tiles % tpg == 0
    gcols = tpg * K2 * emb  # G columns per gather
    tstride = K2 * emb  # per-tile stride inside G
    sstride = K * emb  # pos->neg stride inside a tile block

    with tile.TileContext(nc) as tc:
        with (
            tc.tile_pool(name="idx", bufs=1) as idxp,
            tc.tile_pool(name="g", bufs=g_bufs) as gp,
            tc.psum_pool(name="ps", bufs=2) as psp,
            tc.tile_pool(name="scr", bufs=2) as scrp,
            tc.tile_pool(name="s", bufs=1) as sp,
        ):
            IDX = idxp.tile([P, n_tiles * K2], mybir.dt.int32, tag="idx")
            ID = idxp.tile([P, P], table_dt, tag="ident")
            nc.sync.dma_start(out=IDX[:], in_=idx_in[:])
            nc.sync.dma_start(out=ID[:], in_=ident[:])

            S_pos = sp.tile([P, n_tiles], mybir.dt.float32, tag="spos")
            S_neg = sp.tile([P, n_tiles], mybir.dt.float32, tag="sneg")
            if mode == "dma_only":
                nc.vector.memset(S_pos[:], 0.0)
                nc.vector.memset(S_neg[:], 0.0)

            for _rep in range(reps):
              for t in range(n_tiles // tpg):
                if mode == "compute_only":
                    if "G_static" not in locals():
                        G_static = gp.tile([P, gcols], table_dt, tag="g")
                        nc.gpsimd.indirect_dma_start(
                            out=G_static[:],
                            out_offset=None,
                            in_=table[:],
                            in_offset=bass.IndirectOffsetOnAxis(
                                ap=IDX[:, 0 : tpg * K2], axis=0
                            ),
                        )
                    G = G_static
                else:
                    G = gp.tile([P, gcols], table_dt, tag="g")
                    nc.gpsimd.indirect_dma_start(
                        out=G[:],
                        out_offset=None,
                        in_=table[:],
                        in_offset=bass.IndirectOffsetOnAxis(
                            ap=IDX[:, t * tpg * K2 : (t + 1) * tpg * K2], axis=0
                        ),
                    )
                if mode == "dma_only":
                    # cheap consumer so the gathers aren't dead-store
                    # eliminated
                    JDMA = scrp.tile([P, 32], mybir.dt.float32, tag="jdma")
                    nc.scalar.activation(
                        out=JDMA[:],
                        in_=G[:, 0:32],
                        func=mybir.ActivationFunctionType.Copy,
                        accum_out=S_pos[:, 0:1],
                    )
                    continue

                # 4 ctx-sum groups (tile j x pos/neg) -> one PSUM tile
                PU = psp.tile(
                    [P, 2 * tpg * emb], mybir.dt.float32, tag="pu", space="PSUM"
                )
                n_mm = pe_ctx if pe_ctx is not None else ctx
                for j in range(tpg):
                    for side in range(2):
                        base = j * tstride + side * sstride
                        chunk = (2 * j + side) * emb
                        for c in range(n_mm):
                            nc.tensor.matmul(
                                PU[:, chunk : chunk + emb],
                                ID[:],
                                G[:, base + c * emb : base + (c + 1) * emb],
                                start=(c == 0),
                                stop=(c == n_mm - 1),
                            )

                # one DVE mult for all 4 chunks; in1 = strided view of the
                # target rows: [tile, side, e] with strides [tstride,
                # sstride, 1] starting at ctx*emb
                if skip_mult:
                    RED = PU
                else:
                    DOT = scrp.tile([P, 2 * tpg * emb], mybir.dt.bfloat16, tag="dot")
                    W = G[:].rearrange("p (t s x) -> p t s x", t=tpg, s=2)[
                        :, :, :, ctx * emb : (ctx + 1) * emb
                    ]
                    nc.vector.tensor_tensor(
                        out=DOT[:],
                        in0=PU[:],
                        in1=W,
                        op=mybir.AluOpType.mult,
                    )
                    RED = DOT

                for j in range(tpg):
                    for side in range(2):
                        gt = t * tpg + j
                        S = S_pos if side == 0 else S_neg
                        chunk = (2 * j + side) * emb
                        r_idx = 2 * j + side
                        if r_idx < dve_reduce:
                            JV = scrp.tile([P, emb], mybir.dt.bfloat16, tag="jv")
                            nc.vector.tensor_scalar(
                                out=JV[:],
                                in0=RED[:, chunk : chunk + emb],
                                scalar1=1.0,
                                scalar2=None,
                                op0=mybir.AluOpType.mult,
                                accum_out=S[:, gt : gt + 1],
                            )
                        else:
                            JD = scrp.tile([P, emb], mybir.dt.float32, tag="jd")
                            nc.scalar.activation(
                                out=JD[:],
                                in_=RED[:, chunk : chunk + emb],
                                func=mybir.ActivationFunctionType.Copy,
                                accum_out=S[:, gt : gt + 1],
                            )

            # -log_sigmoid(p) = softplus(-p) = ln(1 + exp(-p));
            # -log_sigmoid(-n) = softplus(n) = ln(1 + exp(n)).
            # Scores carry the 2^20 table scale; descale on Exp's scale input.
            EP = sp.tile([P, n_tiles], mybir.dt.float32, tag="ep")
            EN = sp.tile([P, n_tiles], mybir.dt.float32, tag="en")
            JUNK = sp.tile([P, n_tiles], mybir.dt.float32, tag="junk")
            ACC = sp.tile([P, 2], mybir.dt.float32, tag="acc")
            nc.scalar.activation(
                out=EP[:],
                in_=S_pos[:],
                func=mybir.ActivationFunctionType.Exp,
                scale=-SCORE_DESCALE,
            )
            nc.scalar.activation(
                out=JUNK[:],
                in_=EP[:],
                func=mybir.ActivationFunctionType.Ln,
                bias=1.0,
                accum_out=ACC[:, 0:1],
            )
            nc.scalar.activation(
                out=EN[:],
                in_=S_neg[:],
                func=mybir.ActivationFunctionType.Exp,
                scale=SCORE_DESCALE,
            )
            nc.scalar.activation(
                out=JUNK[:],
                in_=EN[:],
                func=mybir.ActivationFunctionType.Ln,
                bias=1.0,
                accum_out=ACC[:, 1:2],
            )
            V = sp.tile([P, 1], mybir.dt.float32, tag="v")
            nc.vector.tensor_tensor(
                out=V[:],
                in0=ACC[:, 0:1],
                in1=ACC[:, 1:2],
                op=mybir.AluOpType.add,
            )
            nc.sync.dma_start(out=out[:], in_=V[:, 0])
    if finalize:
        nc.finalize()
    return nc


def _prep_idx(pos_u, pos_w, neg_u, neg_w, n_tiles):
    """-> [P, n_tiles*K2] int32, tile-major: partition p, cols t*K2:(t+1)*K2
    hold row t*P+p's [10 pos ctx, pos w+TABLE, 10 neg ctx, neg w+TABLE]."""
    a = np.concatenate(
        [
            pos_u.astype(np.int64),
            (pos_w.astype(np.int64) + TABLE)[:, None],
            neg_u.astype(np.int64),
            (neg_w.astype(np.int64) + TABLE)[:, None],
        ],
        axis=1,
    ).astype(np.int32)
    return np.ascontiguousarray(
        a.reshape(n_tiles, P, K2).transpose(1, 0, 2).reshape(P, n_tiles * K2)
    )


def _prep_table(u_weight, w_weight):
    table_np_dt = mybir.dt.np(mybir.dt.float8e4)
    return np.ascontiguousarray(
        (
            np.concatenate(
                [
                    np.asarray(u_weight, np.float32),
                    np.asarray(w_weight, np.float32),
                ],
                axis=0,
            )
            * TABLE_SCALE
        ).astype(table_np_dt)
    )


def _prep_ident():
    return np.eye(P, dtype=mybir.dt.np(mybir.dt.float8e4))


def make_in_maps(pos_u, pos_w, neg_u, neg_w, u_weight, w_weight):
    pos_u = np.asarray(pos_u)
    pos_w = np.asarray(pos_w)
    neg_u = np.asarray(neg_u)
    neg_w = np.asarray(neg_w)

    table = _prep_table(u_weight, w_weight)
    ident = _prep_ident()

    in_maps = []
    for c in range(N_CORES):
        sl = slice(c * B_CORE, (c + 1) * B_CORE)
        in_maps.append(
            {
                "table": table,
                "ident": ident,
                "idx": _prep_idx(
                    pos_u[sl], pos_w[sl], neg_u[sl], neg_w[sl], N_TILES
                ),
            }
        )
    return in_maps


def kernel(pos_u, pos_w, neg_u, neg_w, u_weight, w_weight):
    from concourse.bass_utils import run_bass_kernel_spmd

    if "nc" not in _NC_CACHE:
        _NC_CACHE["nc"] = build_nc()
    nc = _NC_CACHE["nc"]

    in_maps = make_in_maps(pos_u, pos_w, neg_u, neg_w, u_weight, w_weight)

    global LAST_RESULT
    res = run_bass_kernel_spmd(nc, in_maps, list(range(N_CORES)))
    LAST_RESULT = res
    total = 0.0
    for r in res.results:
        total += float(np.sum(r["out"].astype(np.float64)))
    return np.float32(total)
